# revision 19
# baseline (speedup 1.0000x reference)
"""HGCN layer (gather + segment_sum, two hops) on 8 Trainium2 NeuronCores.

Strategy
--------
The reference computes, for a bipartite graph with E edges:

    rst  = segment_sum(h_src[edge_src], edge_dst, n_dst) * deg_dst^-1   # fwd
    bsrc = segment_sum(rst[edge_dst],  edge_src, n_src) * deg_src^-1    # bwd
    return (bsrc, rst)

On device, a segment-sum is expressed as a chain of one-hot matmuls:
nodes are packed (host side) into windows of 256 "slots"; edges are
sorted by their key node's slot.  For each 128-edge subtile the device
gathers the 128 feature rows with one indirect DMA, builds the one-hot
selection matrix M[e, j] = (rel_slot[e] == j) with a DVE compare against
an iota row, and accumulates  psum[D, 256] += gathered.T @ M  on the PE.
At the end of a window the [D, 256] accumulator is transposed back to
[256, D], scaled by the degree reciprocal and written out.

Work is sharded across the 8 cores by windows (= contiguous slot
ranges).  The forward output rst is AllGather'ed so every core holds the
full [n_dst_slots, D] table for the backward gather phase.

Host-side preprocessing is index-only: node->slot packing (degree-
balanced so every window has nearly the same edge count), edge sorting,
and degree bincounts.  All feature compute happens on device.
"""

import os
import sys

import numpy as np

for _p in ("/opt/trn_rl_repo", "/root/.axon_site/_ro/trn_rl_repo"):
    if os.path.isdir(_p) and _p not in sys.path:
        sys.path.insert(0, _p)

# ---------------------------------------------------------------- constants
D = 128          # feature dim
P = 128          # partitions / edges per subtile
WIN = 256        # node slots per window (PE moving-dim; >=256 keeps f32r fast)
N_CORES = 8
NORM_2 = -1.0    # exponent on deg_src in the backward normalization

# Full-size problem geometry (hardcoded per spec).
N_SRC = 100000
N_DST = 50000
N_EDGES = 1600000


class Cfg:
    """Geometry of the compiled program (same program on all 8 cores)."""

    def __init__(self, n_src, n_dst, n_edges, WF, TF, WB, TB, G=8,
                 n_cores=N_CORES, use_f32r=True):
        self.n_src = n_src          # rows of the forward gather table
        self.n_dst = n_dst
        self.n_edges = n_edges
        self.WF = WF                # forward windows per core
        self.TF = TF                # 128-edge subtiles per forward window
        self.WB = WB                # backward windows per core
        self.TB = TB
        self.G = G                  # subtiles fetched per indirect DMA
        self.n_cores = n_cores
        self.use_f32r = use_f32r

    @property
    def nwin_f(self):
        return self.WF * self.n_cores

    @property
    def nwin_b(self):
        return self.WB * self.n_cores

    @property
    def dst_slots(self):
        return self.nwin_f * WIN

    @property
    def src_slots(self):
        return self.nwin_b * WIN

    # The forward AllGather is split in two so the first half overlaps
    # the tail of the forward gather stream. rst_full is laid out as
    # [all cores' lo windows][all cores' hi windows].
    @property
    def wf_lo(self):
        return (self.WF + 1) // 2

    def rst_row(self, slot):
        """Map a global dst slot (core-major) to its rst_full row."""
        lo = self.wf_lo * WIN
        per = self.WF * WIN
        c, s = slot // per, slot % per
        return np.where(s < lo, c * lo + s,
                        self.n_cores * lo + c * (per - lo) + (s - lo))

    def key(self):
        return (self.n_src, self.n_dst, self.WF, self.TF, self.WB, self.TB,
                self.G, self.n_cores, self.use_f32r)


# Full-size config: fwd 200 windows * 8192-edge capacity (mean load 8000),
# bwd 392 windows * 4224-edge capacity (mean load 4082).
# G=1: hardware indirect DMA consumes exactly one table-row index per
# SBUF partition per instruction (128 rows / 64KB per gather).
FULL_CFG = Cfg(N_SRC, N_DST, N_EDGES, WF=25, TF=64, WB=49, TB=33, G=1)


# ---------------------------------------------------------------- host side
def _snake_pack(deg, n_win):
    """Assign each node a slot so windows have ~equal total degree.

    Nodes are ranked by degree (desc) and dealt to windows in snake
    order, which equalizes both node count and degree sum per window.
    Returns slot[node] (global slot index, window*WIN + position).
    """
    n = deg.shape[0]
    order = np.argsort(-deg, kind="stable")
    r = np.arange(n)
    k, pos = r // n_win, r % n_win
    win_of_rank = np.where(k % 2 == 0, pos, n_win - 1 - pos)
    win_of_node = np.empty(n, np.int64)
    win_of_node[order] = win_of_rank
    cnt = np.bincount(win_of_node, minlength=n_win)
    if cnt.max() > WIN:
        raise AssertionError(f"window node overflow: {cnt.max()} > {WIN}")
    starts = np.zeros(n_win + 1, np.int64)
    np.cumsum(cnt, out=starts[1:])
    ord2 = np.argsort(win_of_node, kind="stable")
    posw = np.empty(n, np.int64)
    posw[ord2] = np.arange(n) - starts[win_of_node[ord2]]
    return win_of_node * WIN + posw


def _pack_edges(gather_idx, key_slot, n_win, T):
    """Lay edges out as [n_win, 128, 2T] int32 metadata tiles.

    Column t (< T) holds the gather row index of subtile t's edge on
    this partition; column T+t holds the edge's within-window slot as
    float32 bits (-1.0 for padding, which matches no slot).
    """
    n_edges = gather_idx.shape[0]
    win_of_edge = key_slot // WIN
    order = np.argsort(win_of_edge, kind="stable")
    cnt = np.bincount(win_of_edge, minlength=n_win)
    if cnt.max() > T * P:
        raise AssertionError(f"window edge overflow: {cnt.max()} > {T * P}")
    starts = np.zeros(n_win + 1, np.int64)
    np.cumsum(cnt, out=starts[1:])
    w = win_of_edge[order]
    pos = np.arange(n_edges) - starts[w]
    t, p = pos // P, pos % P
    meta = np.zeros((n_win, P, 2 * T), np.int32)
    meta[:, :, T:] = np.float32(-1.0).view(np.int32)
    meta[w, p, t] = gather_idx[order].astype(np.int32)
    meta[w, p, T + t] = (key_slot[order] % WIN).astype(np.float32).view(np.int32)
    return meta


def _pack_recip(vals_slot, n_cores, w_per_core):
    """[n_win*WIN] per-slot scale -> [n_cores, 128, 2*w_per_core] with
    col 2*w+j, row p = value of slot (core base) + w*WIN + j*128 + p."""
    v = vals_slot.reshape(n_cores, w_per_core, 2, P).transpose(0, 3, 1, 2)
    return np.ascontiguousarray(v.reshape(n_cores, P, 2 * w_per_core))


def _fit_cfg(base, edge_src, edge_dst):
    """Size the per-window subtile counts to the actual data (the snake
    packing is tight, so this usually shaves a subtile off each phase).
    The compiled program is cached per (TF, TB)."""
    es = np.asarray(edge_src).astype(np.int64)
    ed = np.asarray(edge_dst).astype(np.int64)
    deg_dst = np.bincount(ed, minlength=base.n_dst).astype(np.int64)
    deg_src = np.bincount(es, minlength=base.n_src).astype(np.int64)
    slot_d = _snake_pack(deg_dst, base.nwin_f)
    slot_s = _snake_pack(deg_src, base.nwin_b)
    tf = int(np.ceil(np.bincount(slot_d[ed] // WIN, minlength=base.nwin_f).max() / P))
    tb = int(np.ceil(np.bincount(slot_s[es] // WIN, minlength=base.nwin_b).max() / P))
    return Cfg(base.n_src, base.n_dst, base.n_edges, WF=base.WF, TF=max(tf, 1),
               WB=base.WB, TB=max(tb, 1), G=base.G, n_cores=base.n_cores,
               use_f32r=base.use_f32r)


def _preprocess(cfg, h_src, edge_src, edge_dst):
    es = np.asarray(edge_src).astype(np.int64)
    ed = np.asarray(edge_dst).astype(np.int64)
    deg_dst = np.bincount(ed, minlength=cfg.n_dst).astype(np.int64)
    deg_src = np.bincount(es, minlength=cfg.n_src).astype(np.int64)

    slot_d = _snake_pack(deg_dst, cfg.nwin_f)   # dst node -> dst slot
    slot_s = _snake_pack(deg_src, cfg.nwin_b)   # src node -> src slot

    meta_f = _pack_edges(es, slot_d[ed], cfg.nwin_f, cfg.TF)
    meta_b = _pack_edges(cfg.rst_row(slot_d[ed]), slot_s[es], cfg.nwin_b, cfg.TB)

    dslot_deg = np.zeros(cfg.dst_slots, np.float32)
    dslot_deg[slot_d] = deg_dst
    sslot_deg = np.zeros(cfg.src_slots, np.float32)
    sslot_deg[slot_s] = deg_src
    recip_f = (1.0 / np.maximum(dslot_deg, 1.0)).astype(np.float32)
    recip_b = (np.maximum(sslot_deg, 1.0) ** np.float32(NORM_2)).astype(np.float32)

    h = np.ascontiguousarray(np.asarray(h_src, dtype=np.float32))
    meta_f = meta_f.reshape(cfg.n_cores, cfg.WF, P, 2 * cfg.TF)
    meta_b = meta_b.reshape(cfg.n_cores, cfg.WB, P, 2 * cfg.TB)
    rec_f = _pack_recip(recip_f, cfg.n_cores, cfg.WF)
    rec_b = _pack_recip(recip_b, cfg.n_cores, cfg.WB)

    in_maps = [
        {
            "h": h,
            "meta_f": np.ascontiguousarray(meta_f[c]),
            "meta_b": np.ascontiguousarray(meta_b[c]),
            "recip_f": rec_f[c],
            "recip_b": rec_b[c],
        }
        for c in range(cfg.n_cores)
    ]
    return in_maps, slot_d, slot_s


# -------------------------------------------------------------- device side
def _build_program(cfg):
    import concourse.bass as bass
    import concourse.tile as tile
    from concourse import bacc, mybir
    from concourse.masks import make_identity
    from contextlib import ExitStack

    f32 = mybir.dt.float32
    f32r = mybir.dt.float32r
    i32 = mybir.dt.int32

    # With use_f32r, every matmul operand (the gathered feature rows and
    # the one-hot M matrix) lives in float32r tiles so the PE takes the
    # full-rate fp32r path and the BIR verifier sees rounded producers.
    # float32r has the same bit layout as float32 on the host side.
    mdt = f32r if cfg.use_f32r else f32

    nc = bacc.Bacc("TRN2", target_bir_lowering=False, debug=False,
                   num_devices=cfg.n_cores)

    h_in = nc.dram_tensor("h", [cfg.n_src, D], mdt, kind="ExternalInput")
    meta_f = nc.dram_tensor("meta_f", [cfg.WF, P, 2 * cfg.TF], i32,
                            kind="ExternalInput")
    meta_b = nc.dram_tensor("meta_b", [cfg.WB, P, 2 * cfg.TB], i32,
                            kind="ExternalInput")
    recip_f = nc.dram_tensor("recip_f", [P, 2 * cfg.WF], f32,
                             kind="ExternalInput")
    recip_b = nc.dram_tensor("recip_b", [P, 2 * cfg.WB], f32,
                             kind="ExternalInput")
    rst_out = nc.dram_tensor("rst_out", [cfg.WF * WIN, D], f32,
                             kind="ExternalOutput")
    bsrc_out = nc.dram_tensor("bsrc_out", [cfg.WB * WIN, D], f32,
                              kind="ExternalOutput")

    lo_rows = cfg.wf_lo * WIN
    hi_rows = (cfg.WF - cfg.wf_lo) * WIN

    with tile.TileContext(nc) as tc, ExitStack() as ctx:
        dram = ctx.enter_context(tc.tile_pool(name="dram", bufs=1, space="DRAM"))
        rst_local_lo = dram.tile([lo_rows, D], mdt)
        rst_local_hi = dram.tile([hi_rows, D], mdt)
        # Local (not Shared): two collectives write disjoint halves, and
        # the shared-scratchpad protocol allows only a single writer.
        rst_full = dram.tile([cfg.dst_slots, D], mdt)

        # Deep gather buffering: the kernel is bound by the GpSimd Q7's
        # ~1.1us fixed cost per indirect DMA, so the gather stream must
        # never stall on a free slot.
        const_pool = ctx.enter_context(tc.tile_pool(name="const", bufs=1))
        meta_pool = ctx.enter_context(tc.tile_pool(name="meta", bufs=4))
        gath_pool = ctx.enter_context(tc.tile_pool(name="gath", bufs=16))
        m_pool = ctx.enter_context(tc.tile_pool(name="mtile", bufs=8))
        acc_pool = ctx.enter_context(tc.tile_pool(name="acc", bufs=3))
        out_pool = ctx.enter_context(tc.tile_pool(name="outp", bufs=4))
        psum_pool = ctx.enter_context(tc.tile_pool(name="psum", bufs=3, space="PSUM"))
        tps_pool = ctx.enter_context(tc.tile_pool(name="tpsum", bufs=2, space="PSUM"))

        iota_i = const_pool.tile([P, WIN], i32)
        nc.gpsimd.iota(iota_i[:], pattern=[[1, WIN]], base=0, channel_multiplier=0)
        iota_f = const_pool.tile([P, WIN], f32)
        nc.vector.tensor_copy(iota_f[:], iota_i[:])
        ident = const_pool.tile([P, P], f32)
        make_identity(nc, ident[:])
        recf = const_pool.tile([P, 2 * cfg.WF], f32)
        nc.sync.dma_start(recf[:], recip_f.ap())
        recb = const_pool.tile([P, 2 * cfg.WB], f32)
        nc.sync.dma_start(recb[:], recip_b.ap())

        def phase(n_win, T, meta_in, table_ap, rec_tile, out_targets):
            for w in range(n_win):
                meta = meta_pool.tile([P, 2 * T], i32, tag="meta")
                nc.sync.dma_start(meta[:], meta_in.ap()[w])
                psum = psum_pool.tile([P, WIN], f32, tag="psum")
                for t0 in range(0, T, cfg.G):
                    gsz = min(cfg.G, T - t0)
                    gath = gath_pool.tile([P, cfg.G * D], mdt, tag="gath")
                    nc.gpsimd.indirect_dma_start(
                        out=gath[:, : gsz * D],
                        out_offset=None,
                        in_=table_ap,
                        in_offset=bass.IndirectOffsetOnAxis(
                            ap=meta[:, t0 : t0 + gsz], axis=0
                        ),
                    )
                    for j in range(gsz):
                        t = t0 + j
                        m = m_pool.tile([P, WIN], mdt, tag="m")
                        nc.vector.tensor_tensor(
                            out=m[:],
                            in0=meta[:, T + t : T + t + 1]
                            .bitcast(f32)
                            .to_broadcast([P, WIN]),
                            in1=iota_f[:],
                            op=mybir.AluOpType.is_equal,
                        )
                        nc.tensor.matmul(
                            out=psum[:],
                            lhsT=gath[:, j * D : (j + 1) * D],
                            rhs=m[:],
                            start=(t == 0),
                            stop=(t == T - 1),
                        )
                acc = acc_pool.tile([P, WIN], f32, tag="acc")
                nc.scalar.copy(acc[:], psum[:])
                for j in range(2):
                    tp = tps_pool.tile([P, P], f32, tag="tp")
                    nc.tensor.transpose(
                        out=tp[:], in_=acc[:, j * P : (j + 1) * P], identity=ident[:]
                    )
                    o = out_pool.tile([P, D], f32, tag="o")
                    nc.vector.tensor_scalar_mul(
                        o[:], tp[:], rec_tile[:, 2 * w + j : 2 * w + j + 1]
                    )
                    for tgt, base in out_targets(w):
                        r0 = base + j * P
                        src = o[:] if tgt.dtype == f32 else o[:].bitcast(tgt.dtype)
                        nc.sync.dma_start(tgt[r0 : r0 + P, :], src)

        def fwd_targets(w):
            tgts = [(rst_out.ap(), w * WIN)]
            if w < cfg.wf_lo:
                tgts.append((rst_local_lo[:], w * WIN))
            else:
                tgts.append((rst_local_hi[:], (w - cfg.wf_lo) * WIN))
            return tgts

        phase(cfg.WF, cfg.TF, meta_f, h_in.ap(), recf, fwd_targets)
        # Two-part AllGather: the lo half only depends on windows
        # [0, wf_lo) and overlaps the tail of the forward gather stream.
        groups = [list(range(cfg.n_cores))]
        nc.gpsimd.collective_compute(
            "AllGather", mybir.AluOpType.bypass, replica_groups=groups,
            ins=[rst_local_lo[:].opt()],
            outs=[rst_full[: cfg.n_cores * lo_rows, :].opt()],
        )
        nc.gpsimd.collective_compute(
            "AllGather", mybir.AluOpType.bypass, replica_groups=groups,
            ins=[rst_local_hi[:].opt()],
            outs=[rst_full[cfg.n_cores * lo_rows :, :].opt()],
        )
        phase(cfg.WB, cfg.TB, meta_b, rst_full[:], recb,
              lambda w: [(bsrc_out.ap(), w * WIN)])

    nc.compile()
    return nc


_PROGRAM_CACHE = {}


def _get_program(cfg):
    key = cfg.key()
    if key not in _PROGRAM_CACHE:
        _PROGRAM_CACHE[key] = _build_program(cfg)
    return _PROGRAM_CACHE[key]


LAST_EXEC_NS = None
LAST_TRACE = None


def _run(cfg, h_src, edge_src, edge_dst, trace=False):
    global LAST_EXEC_NS, LAST_TRACE
    from concourse.bass_utils import run_bass_kernel_spmd

    nc = _get_program(cfg)
    in_maps, slot_d, slot_s = _preprocess(cfg, h_src, edge_src, edge_dst)
    res = run_bass_kernel_spmd(nc, in_maps, list(range(cfg.n_cores)),
                               trace=trace)
    LAST_EXEC_NS = res.exec_time_ns
    LAST_TRACE = res.instructions_and_trace
    rst_slots = np.concatenate([r["rst_out"] for r in res.results], axis=0)
    bsrc_slots = np.concatenate([r["bsrc_out"] for r in res.results], axis=0)
    rst = rst_slots[slot_d]
    bsrc = bsrc_slots[slot_s]
    return bsrc, rst


def kernel(h_src, edge_src, edge_dst, n_dst):
    h = np.asarray(h_src, dtype=np.float32)
    assert h.shape == (N_SRC, D), f"unexpected h_src shape {h.shape}"
    assert int(np.asarray(n_dst)) == N_DST
    trace = bool(int(os.environ.get("GNN_KERNEL_TRACE", "0")))
    cfg = _fit_cfg(FULL_CFG, edge_src, edge_dst)
    bsrc, rst = _run(cfg, h, edge_src, edge_dst, trace=trace)
    return bsrc, rst


# revision 23
# speedup vs baseline: 1.0125x; 1.0125x over previous
"""HGCN layer (gather + segment_sum, two hops) on 8 Trainium2 NeuronCores.

Strategy
--------
The reference computes, for a bipartite graph with E edges:

    rst  = segment_sum(h_src[edge_src], edge_dst, n_dst) * deg_dst^-1   # fwd
    bsrc = segment_sum(rst[edge_dst],  edge_src, n_src) * deg_src^-1    # bwd
    return (bsrc, rst)

On device, a segment-sum is expressed as a chain of one-hot matmuls:
nodes are packed (host side) into windows of 256 "slots"; edges are
sorted by their key node's slot.  For each 128-edge subtile the device
gathers the 128 feature rows with one indirect DMA, builds the one-hot
selection matrix M[e, j] = (rel_slot[e] == j) with a DVE compare against
an iota row, and accumulates  psum[D, 256] += gathered.T @ M  on the PE.
At the end of a window the [D, 256] accumulator is transposed back to
[256, D], scaled by the degree reciprocal and written out.

Work is sharded across the 8 cores by windows (= contiguous slot
ranges).  The forward output rst is AllGather'ed so every core holds the
full [n_dst_slots, D] table for the backward gather phase.

Host-side preprocessing is index-only: node->slot packing (degree-
balanced so every window has nearly the same edge count), edge sorting,
and degree bincounts.  All feature compute happens on device.
"""

import os
import sys

import numpy as np

for _p in ("/opt/trn_rl_repo", "/root/.axon_site/_ro/trn_rl_repo"):
    if os.path.isdir(_p) and _p not in sys.path:
        sys.path.insert(0, _p)

# ---------------------------------------------------------------- constants
D = 128          # feature dim
P = 128          # partitions / edges per subtile
WIN = 256        # node slots per window (PE moving-dim; >=256 keeps f32r fast)
N_CORES = 8
NORM_2 = -1.0    # exponent on deg_src in the backward normalization

# Full-size problem geometry (hardcoded per spec).
N_SRC = 100000
N_DST = 50000
N_EDGES = 1600000


class Cfg:
    """Geometry of the compiled program (same program on all 8 cores)."""

    def __init__(self, n_src, n_dst, n_edges, WF, TF, WB, TB, G=8,
                 n_cores=N_CORES, use_f32r=True):
        self.n_src = n_src          # rows of the forward gather table
        self.n_dst = n_dst
        self.n_edges = n_edges
        self.WF = WF                # forward windows per core
        self.TF = TF                # 128-edge subtiles per forward window
        self.WB = WB                # backward windows per core
        self.TB = TB
        self.G = G                  # subtiles fetched per indirect DMA
        self.n_cores = n_cores
        self.use_f32r = use_f32r

    @property
    def nwin_f(self):
        return self.WF * self.n_cores

    @property
    def nwin_b(self):
        return self.WB * self.n_cores

    @property
    def dst_slots(self):
        return self.nwin_f * WIN

    @property
    def src_slots(self):
        return self.nwin_b * WIN

    # The forward AllGather is split in two so the first half overlaps
    # the tail of the forward gather stream. rst_full is laid out as
    # [all cores' lo windows][all cores' hi windows].
    @property
    def wf_lo(self):
        return (self.WF + 1) // 2

    def rst_row(self, slot):
        """Map a global dst slot (core-major) to its rst_full row."""
        lo = self.wf_lo * WIN
        per = self.WF * WIN
        c, s = slot // per, slot % per
        return np.where(s < lo, c * lo + s,
                        self.n_cores * lo + c * (per - lo) + (s - lo))

    def key(self):
        return (self.n_src, self.n_dst, self.WF, self.TF, self.WB, self.TB,
                self.G, self.n_cores, self.use_f32r)


# Full-size config: fwd 200 windows * 8192-edge capacity (mean load 8000),
# bwd 392 windows * 4224-edge capacity (mean load 4082).
# G=1: hardware indirect DMA consumes exactly one table-row index per
# SBUF partition per instruction (128 rows / 64KB per gather).
FULL_CFG = Cfg(N_SRC, N_DST, N_EDGES, WF=25, TF=64, WB=49, TB=33, G=1)


# ---------------------------------------------------------------- host side
def _snake_pack(deg, n_win):
    """Assign each node a slot so windows have ~equal total degree.

    Nodes are ranked by degree (desc) and dealt to windows in snake
    order, which equalizes both node count and degree sum per window.
    Returns slot[node] (global slot index, window*WIN + position).
    """
    n = deg.shape[0]
    order = np.argsort(-deg, kind="stable")
    r = np.arange(n)
    k, pos = r // n_win, r % n_win
    win_of_rank = np.where(k % 2 == 0, pos, n_win - 1 - pos)
    win_of_node = np.empty(n, np.int64)
    win_of_node[order] = win_of_rank
    cnt = np.bincount(win_of_node, minlength=n_win)
    if cnt.max() > WIN:
        raise AssertionError(f"window node overflow: {cnt.max()} > {WIN}")
    starts = np.zeros(n_win + 1, np.int64)
    np.cumsum(cnt, out=starts[1:])
    ord2 = np.argsort(win_of_node, kind="stable")
    posw = np.empty(n, np.int64)
    posw[ord2] = np.arange(n) - starts[win_of_node[ord2]]
    return win_of_node * WIN + posw


def _pack_edges(gather_idx, key_slot, n_win, T):
    """Lay edges out as [n_win, 128, 2T] int32 metadata tiles.

    Column t (< T) holds the gather row index of subtile t's edge on
    this partition; column T+t holds the edge's within-window slot as
    float32 bits (-1.0 for padding, which matches no slot).
    """
    n_edges = gather_idx.shape[0]
    win_of_edge = key_slot // WIN
    order = np.argsort(win_of_edge, kind="stable")
    cnt = np.bincount(win_of_edge, minlength=n_win)
    if cnt.max() > T * P:
        raise AssertionError(f"window edge overflow: {cnt.max()} > {T * P}")
    starts = np.zeros(n_win + 1, np.int64)
    np.cumsum(cnt, out=starts[1:])
    w = win_of_edge[order]
    pos = np.arange(n_edges) - starts[w]
    t, p = pos // P, pos % P
    meta = np.zeros((n_win, P, 2 * T), np.int32)
    meta[:, :, T:] = np.float32(-1.0).view(np.int32)
    meta[w, p, t] = gather_idx[order].astype(np.int32)
    meta[w, p, T + t] = (key_slot[order] % WIN).astype(np.float32).view(np.int32)
    return meta


def _pack_recip(vals_slot, n_cores, w_per_core):
    """[n_win*WIN] per-slot scale -> [n_cores, 128, 2*w_per_core] with
    col 2*w+j, row p = value of slot (core base) + w*WIN + j*128 + p."""
    v = vals_slot.reshape(n_cores, w_per_core, 2, P).transpose(0, 3, 1, 2)
    return np.ascontiguousarray(v.reshape(n_cores, P, 2 * w_per_core))


def _fit_cfg(base, edge_src, edge_dst):
    """Size the per-window subtile counts to the actual data (the snake
    packing is tight, so this usually shaves a subtile off each phase).
    The compiled program is cached per (TF, TB)."""
    es = np.asarray(edge_src).astype(np.int64)
    ed = np.asarray(edge_dst).astype(np.int64)
    deg_dst = np.bincount(ed, minlength=base.n_dst).astype(np.int64)
    deg_src = np.bincount(es, minlength=base.n_src).astype(np.int64)
    slot_d = _snake_pack(deg_dst, base.nwin_f)
    slot_s = _snake_pack(deg_src, base.nwin_b)
    tf = int(np.ceil(np.bincount(slot_d[ed] // WIN, minlength=base.nwin_f).max() / P))
    tb = int(np.ceil(np.bincount(slot_s[es] // WIN, minlength=base.nwin_b).max() / P))
    return Cfg(base.n_src, base.n_dst, base.n_edges, WF=base.WF, TF=max(tf, 1),
               WB=base.WB, TB=max(tb, 1), G=base.G, n_cores=base.n_cores,
               use_f32r=base.use_f32r)


def _preprocess(cfg, h_src, edge_src, edge_dst):
    es = np.asarray(edge_src).astype(np.int64)
    ed = np.asarray(edge_dst).astype(np.int64)
    deg_dst = np.bincount(ed, minlength=cfg.n_dst).astype(np.int64)
    deg_src = np.bincount(es, minlength=cfg.n_src).astype(np.int64)

    slot_d = _snake_pack(deg_dst, cfg.nwin_f)   # dst node -> dst slot
    slot_s = _snake_pack(deg_src, cfg.nwin_b)   # src node -> src slot

    meta_f = _pack_edges(es, slot_d[ed], cfg.nwin_f, cfg.TF)
    meta_b = _pack_edges(slot_d[ed], slot_s[es], cfg.nwin_b, cfg.TB)

    dslot_deg = np.zeros(cfg.dst_slots, np.float32)
    dslot_deg[slot_d] = deg_dst
    sslot_deg = np.zeros(cfg.src_slots, np.float32)
    sslot_deg[slot_s] = deg_src
    recip_f = (1.0 / np.maximum(dslot_deg, 1.0)).astype(np.float32)
    recip_b = (np.maximum(sslot_deg, 1.0) ** np.float32(NORM_2)).astype(np.float32)

    h = np.ascontiguousarray(np.asarray(h_src, dtype=np.float32))
    meta_f = meta_f.reshape(cfg.n_cores, cfg.WF, P, 2 * cfg.TF)
    meta_b = meta_b.reshape(cfg.n_cores, cfg.WB, P, 2 * cfg.TB)
    rec_f = _pack_recip(recip_f, cfg.n_cores, cfg.WF)
    rec_b = _pack_recip(recip_b, cfg.n_cores, cfg.WB)

    in_maps = [
        {
            "h": h,
            "meta_f": np.ascontiguousarray(meta_f[c]),
            "meta_b": np.ascontiguousarray(meta_b[c]),
            "recip_f": rec_f[c],
            "recip_b": rec_b[c],
        }
        for c in range(cfg.n_cores)
    ]
    return in_maps, slot_d, slot_s


# -------------------------------------------------------------- device side
def _build_program(cfg):
    import concourse.bass as bass
    import concourse.tile as tile
    from concourse import bacc, mybir
    from concourse.masks import make_identity
    from contextlib import ExitStack

    f32 = mybir.dt.float32
    f32r = mybir.dt.float32r
    i32 = mybir.dt.int32

    # With use_f32r, every matmul operand (the gathered feature rows and
    # the one-hot M matrix) lives in float32r tiles so the PE takes the
    # full-rate fp32r path and the BIR verifier sees rounded producers.
    # float32r has the same bit layout as float32 on the host side.
    mdt = f32r if cfg.use_f32r else f32

    nc = bacc.Bacc("TRN2", target_bir_lowering=False, debug=False,
                   num_devices=cfg.n_cores)

    h_in = nc.dram_tensor("h", [cfg.n_src, D], mdt, kind="ExternalInput")
    meta_f = nc.dram_tensor("meta_f", [cfg.WF, P, 2 * cfg.TF], i32,
                            kind="ExternalInput")
    meta_b = nc.dram_tensor("meta_b", [cfg.WB, P, 2 * cfg.TB], i32,
                            kind="ExternalInput")
    recip_f = nc.dram_tensor("recip_f", [P, 2 * cfg.WF], f32,
                             kind="ExternalInput")
    recip_b = nc.dram_tensor("recip_b", [P, 2 * cfg.WB], f32,
                             kind="ExternalInput")
    rst_out = nc.dram_tensor("rst_out", [cfg.WF * WIN, D], f32,
                             kind="ExternalOutput")
    bsrc_out = nc.dram_tensor("bsrc_out", [cfg.WB * WIN, D], f32,
                              kind="ExternalOutput")

    with tile.TileContext(nc) as tc, ExitStack() as ctx:
        dram = ctx.enter_context(tc.tile_pool(name="dram", bufs=1, space="DRAM"))
        rst_local = dram.tile([cfg.WF * WIN, D], mdt)
        rst_full = dram.tile([cfg.dst_slots, D], mdt, addr_space="Shared")

        # Deep gather buffering: the kernel is bound by the GpSimd Q7's
        # ~1.1us fixed cost per indirect DMA, so the gather stream must
        # never stall on a free slot.
        const_pool = ctx.enter_context(tc.tile_pool(name="const", bufs=1))
        meta_pool = ctx.enter_context(tc.tile_pool(name="meta", bufs=4))
        gath_pool = ctx.enter_context(tc.tile_pool(name="gath", bufs=16))
        m_pool = ctx.enter_context(tc.tile_pool(name="mtile", bufs=8))
        acc_pool = ctx.enter_context(tc.tile_pool(name="acc", bufs=3))
        out_pool = ctx.enter_context(tc.tile_pool(name="outp", bufs=4))
        psum_pool = ctx.enter_context(tc.tile_pool(name="psum", bufs=3, space="PSUM"))
        tps_pool = ctx.enter_context(tc.tile_pool(name="tpsum", bufs=2, space="PSUM"))

        iota_i = const_pool.tile([P, WIN], i32)
        nc.gpsimd.iota(iota_i[:], pattern=[[1, WIN]], base=0, channel_multiplier=0)
        iota_f = const_pool.tile([P, WIN], f32)
        nc.vector.tensor_copy(iota_f[:], iota_i[:])
        ident = const_pool.tile([P, P], f32)
        make_identity(nc, ident[:])
        recf = const_pool.tile([P, 2 * cfg.WF], f32)
        nc.sync.dma_start(recf[:], recip_f.ap())
        recb = const_pool.tile([P, 2 * cfg.WB], f32)
        nc.sync.dma_start(recb[:], recip_b.ap())

        def phase(n_win, T, meta_in, table_ap, rec_tile, out_targets):
            for w in range(n_win):
                meta = meta_pool.tile([P, 2 * T], i32, tag="meta")
                nc.sync.dma_start(meta[:], meta_in.ap()[w])
                psum = psum_pool.tile([P, WIN], f32, tag="psum")
                for t0 in range(0, T, cfg.G):
                    gsz = min(cfg.G, T - t0)
                    gath = gath_pool.tile([P, cfg.G * D], mdt, tag="gath")
                    nc.gpsimd.indirect_dma_start(
                        out=gath[:, : gsz * D],
                        out_offset=None,
                        in_=table_ap,
                        in_offset=bass.IndirectOffsetOnAxis(
                            ap=meta[:, t0 : t0 + gsz], axis=0
                        ),
                    )
                    for j in range(gsz):
                        t = t0 + j
                        m = m_pool.tile([P, WIN], mdt, tag="m")
                        nc.vector.tensor_tensor(
                            out=m[:],
                            in0=meta[:, T + t : T + t + 1]
                            .bitcast(f32)
                            .to_broadcast([P, WIN]),
                            in1=iota_f[:],
                            op=mybir.AluOpType.is_equal,
                        )
                        nc.tensor.matmul(
                            out=psum[:],
                            lhsT=gath[:, j * D : (j + 1) * D],
                            rhs=m[:],
                            start=(t == 0),
                            stop=(t == T - 1),
                        )
                acc = acc_pool.tile([P, WIN], f32, tag="acc")
                nc.scalar.copy(acc[:], psum[:])
                for j in range(2):
                    tp = tps_pool.tile([P, P], f32, tag="tp")
                    nc.tensor.transpose(
                        out=tp[:], in_=acc[:, j * P : (j + 1) * P], identity=ident[:]
                    )
                    o = out_pool.tile([P, D], f32, tag="o")
                    nc.vector.tensor_scalar_mul(
                        o[:], tp[:], rec_tile[:, 2 * w + j : 2 * w + j + 1]
                    )
                    for tgt, base in out_targets(w):
                        r0 = base + j * P
                        src = o[:] if tgt.dtype == f32 else o[:].bitcast(tgt.dtype)
                        nc.sync.dma_start(tgt[r0 : r0 + P, :], src)

        def fwd_targets(w):
            return [(rst_out.ap(), w * WIN), (rst_local[:], w * WIN)]

        phase(cfg.WF, cfg.TF, meta_f, h_in.ap(), recf, fwd_targets)
        nc.gpsimd.collective_compute(
            "AllGather",
            mybir.AluOpType.bypass,
            replica_groups=[list(range(cfg.n_cores))],
            ins=[rst_local[:].opt()],
            outs=[rst_full[:].opt()],
        )
        phase(cfg.WB, cfg.TB, meta_b, rst_full[:], recb,
              lambda w: [(bsrc_out.ap(), w * WIN)])

    nc.compile()
    return nc


_PROGRAM_CACHE = {}


def _get_program(cfg):
    key = cfg.key()
    if key not in _PROGRAM_CACHE:
        _PROGRAM_CACHE[key] = _build_program(cfg)
    return _PROGRAM_CACHE[key]


LAST_EXEC_NS = None
LAST_TRACE = None


def _run(cfg, h_src, edge_src, edge_dst, trace=False):
    global LAST_EXEC_NS, LAST_TRACE
    from concourse.bass_utils import run_bass_kernel_spmd

    nc = _get_program(cfg)
    in_maps, slot_d, slot_s = _preprocess(cfg, h_src, edge_src, edge_dst)
    res = run_bass_kernel_spmd(nc, in_maps, list(range(cfg.n_cores)),
                               trace=trace)
    LAST_EXEC_NS = res.exec_time_ns
    LAST_TRACE = res.instructions_and_trace
    rst_slots = np.concatenate([r["rst_out"] for r in res.results], axis=0)
    bsrc_slots = np.concatenate([r["bsrc_out"] for r in res.results], axis=0)
    rst = rst_slots[slot_d]
    bsrc = bsrc_slots[slot_s]
    return bsrc, rst


def kernel(h_src, edge_src, edge_dst, n_dst):
    h = np.asarray(h_src, dtype=np.float32)
    assert h.shape == (N_SRC, D), f"unexpected h_src shape {h.shape}"
    assert int(np.asarray(n_dst)) == N_DST
    trace = bool(int(os.environ.get("GNN_KERNEL_TRACE", "0")))
    cfg = _fit_cfg(FULL_CFG, edge_src, edge_dst)
    bsrc, rst = _run(cfg, h, edge_src, edge_dst, trace=trace)
    return bsrc, rst
